# revision 33
# baseline (speedup 1.0000x reference)
"""Trainium2 Bass kernel: multi-scale masked average-pool descriptors.

Computes, per batch element b and scribble i:
    d_l[b,i,c] = mean over {pixels where resize(scribble)[b,i,y,x] > 0.5} of feat_l[b,c,y,x]
    out[b,i,c] = (d_0 + d_1 + d_2) / 3

Strategy (v5 -- all constants below are HW-measured):
  * jax.image.resize(bilinear, antialias=False) at scales 4/8/16 reduces to an
    exact 2x2 average at stride k with offset o (k,o) = (4,1)/(8,3)/(16,7):
    mask == ((a+c)+(b+d)) > 2.0 bit-exactly in fp32 (computed on DVE, only
    the 2-of-k needed columns).  Scribbles ride the gpsimd SWDGE queue as
    merged 4KB row-pair descriptors.
  * Level 0 features (16.8MB, the bulk) are DMA'd with FULL-ROW descriptors
    ([y, 16c, 128x] tiles, one 512B descriptor per (c,y) row) split across
    the two HWDGE rings -- the descriptor walk performs the [c,y,x]->[y,..]
    partition transpose for free at ~233 GB/s.  Engine copies (cross-assigned
    scalar/DVE) re-pack each group into one assembled [y, x, 256c] bf16 tile
    so each of the 128 x-column matmuls has a contiguous [128, 256] rhs
    (251ns/LDW+MM pair measured; strided rhs would be 3x slower).  These
    matmuls are emitted LAST: they are the only consumer of the final DMA
    bytes and form the ~20us kernel tail.
  * Levels 1/2 features load in native [c, s] layout (contiguous 8-16KB
    descriptors, bf16-cast inline by the gpsimd SWDGE), are transposed by
    the PE (128x128 identity matmuls into PSUM, batched 4 blocks/bank) into
    raster [s, c] tiles, and run one matmul per 128-pixel raster group --
    their whole pipeline hides inside level 0's DMA window.
  * L1/L2 masks are built in a partition fold (q=32*ihi+g / q=8*ihi+g) such
    that a PE transpose of each ilo-slice directly yields the matmul lhsT
    [128px, 16img] (image order comes out permuted; the host un-permutes).
  * cnt[i] = reduce_sum over the mask + a ones-matmul; bf16 masks are exact
    0/1 and PSUM accumulates fp32, so cnt is exact and masks match the
    reference bit-exactly.  bf16 features give rel err ~2e-3 (gate: 2e-2).
  * The empty-mask fallback is handled on the host (P(empty) ~ 2^-1024).

Sharding: pure data-parallel over batch B=8 across the 8 NeuronCores.
"""

import numpy as np

_B = 8
_I = 16
_C = 256

# level: (h, k, off)
_LEVELS = {0: (128, 4, 1), 1: (64, 8, 3), 2: (32, 16, 7)}

# PE-transpose-induced image permutation (levels 1/2): matmul M position of
# image i.  i = ihi*(16/hh) + ilo  ->  b = ilo*hh + ihi  (hh = 2 for L1, 4
# for L2).
_IMG2POS = {
    1: [(i % 8) * 2 + i // 8 for i in range(16)],
    2: [(i % 4) * 4 + i // 4 for i in range(16)],
}


def _build_nc():
    import concourse.bacc as bacc
    import concourse.tile as tile
    from concourse import mybir

    f32 = mybir.dt.float32
    bf16 = mybir.dt.bfloat16
    gt = mybir.AluOpType.is_gt
    X = mybir.AxisListType.X

    nc = bacc.Bacc("TRN2", target_bir_lowering=False, debug=False)

    feats = {
        0: nc.dram_tensor("feat0", [_C, 128, 128], f32, kind="ExternalInput"),
        1: nc.dram_tensor("feat1", [_C, 64, 64], f32, kind="ExternalInput"),
        2: nc.dram_tensor("feat2", [_C, 32, 32], f32, kind="ExternalInput"),
    }
    scr = nc.dram_tensor("scribbles", [_I, 512, 512], f32, kind="ExternalInput")
    ident_d = nc.dram_tensor("ident", [128, 128], bf16, kind="ExternalInput")
    out_d = nc.dram_tensor("out", [_I, 3 * (_C + 1)], f32, kind="ExternalOutput")

    with tile.TileContext(nc) as tc:
        with (
            tc.tile_pool(name="singles", bufs=1) as singles,
            tc.tile_pool(name="scrib", bufs=2) as scrib,
            tc.tile_pool(name="tmp", bufs=2) as tmp,
            tc.tile_pool(name="fR", bufs=4) as fR,
            tc.tile_pool(name="fc", bufs=1) as fc,
            tc.tile_pool(name="psum", bufs=2, space="PSUM") as psum,
        ):
            ones = singles.tile([128, 1], f32, tag="ones")
            nc.vector.memset(ones[:], 1.0)
            stag = singles.tile([_I, 3 * (_C + 1)], f32, tag="stag")
            ident = singles.tile([128, 128], bf16, tag="ident")
            nc.sync.dma_start(out=ident[:], in_=ident_d[:])

            # masks. L0: y-layout [y, i, x].  L1/L2: A-form fold
            # msk_l[q, ilo, ysub, x] with q = (32|8)*ihi + g, y = hh*g + ysub
            msk0 = singles.tile([128, _I, 128], bf16, tag="msk0")
            msk1 = singles.tile([64, 8, 2, 64], bf16, tag="msk1")
            msk2 = singles.tile([32, 4, 4, 32], bf16, tag="msk2")
            # transposed lhsT tiles [128px, 16b, G]
            mT1 = singles.tile([128, _I, 32], bf16, tag="mT1")
            mT2 = singles.tile([128, _I, 8], bf16, tag="mT2")
            # feature tiles: L0 assembled [y, x, c]; L1/L2 raster [s, g, H, c]
            sgT0 = singles.tile([128, 128, _C], bf16, tag="sgT0")
            T1 = singles.tile([128, 32, 2, 128], bf16, tag="T1")
            T2 = singles.tile([128, 8, 2, 128], bf16, tag="T2")

            # ---- L1/L2 feature cast-loads (gpsimd, contiguous descriptors)
            sgc = {}
            for li, S in ((1, 4096), (2, 1024)):
                for H in range(2):
                    sg = fc.tile([128, S], bf16, tag=f"sgc{li}{H}")
                    nc.gpsimd.dma_start(
                        out=sg[:],
                        in_=feats[li].rearrange("c y x -> c (y x)")[
                            128 * H : 128 * (H + 1), :
                        ],
                    )
                    sgc[(li, H)] = sg

            # ---- L1/L2 feature PE transposes, batched 4 blocks per PSUM
            # bank; drains alternate DVE/gpsimd
            nbat = [0]
            for li, T in ((1, T1), (2, T2)):
                S = 4096 if li == 1 else 1024
                for H in range(2):
                    sg = sgc[(li, H)]
                    for q in range(S // 512):
                        psB = psum.tile([128, 512], bf16, tag="psB")
                        for b_ in range(4):
                            blk = 4 * q + b_
                            nc.tensor.transpose(
                                psB[:, 128 * b_ : 128 * (b_ + 1)],
                                sg[:, 128 * blk : 128 * (blk + 1)],
                                ident[:],
                            )
                        dstT = T[:, 4 * q : 4 * q + 4, H, :]
                        srcB = psB[:].rearrange("p (g c) -> p g c", g=4)
                        if nbat[0] % 2 == 0:
                            nc.vector.tensor_copy(dstT, srcB)
                        else:
                            nc.scalar.copy(dstT, srcB)
                        nbat[0] += 1

            # ---- mask builders ------------------------------------------
            def mask_ops(li, out_ap, src_lo, src_hi, h, w):
                k, off = _LEVELS[li][1], _LEVELS[li][2]
                a = src_lo.rearrange("p (x k) -> p x k", k=k)[:, :, off : off + 2]
                b = src_hi.rearrange("p (x k) -> p x k", k=k)[:, :, off : off + 2]
                v = tmp.tile([h, w, 2], f32, tag=f"v{li}")
                nc.vector.tensor_add(v[:], a, b)
                sr = tmp.tile([h, w], f32, tag=f"sr{li}")
                nc.vector.tensor_add(sr[:], v[:, :, 0], v[:, :, 1])
                nc.vector.tensor_scalar(
                    out=out_ap, in0=sr[:], scalar1=2.0, scalar2=None, op0=gt
                )

            def feat_load_copy(g):
                # L0: 16-channel full-row staging groups; loads alternate the
                # two HWDGE rings (descriptor gen is ~1.4ns/desc); copies are
                # cross-assigned (scalar copies what sync loaded, DVE copies
                # what scalar loaded) so a copy never waits on its own
                # engine's queue head.
                sg = fR.tile([128, 16, 128], f32, tag="sgR")
                deng = nc.sync if g % 2 == 0 else nc.scalar
                deng.dma_start(
                    out=sg[:],
                    in_=feats[0][16 * g : 16 * (g + 1)].rearrange(
                        "c y x -> y c x"
                    ),
                )
                dst = sgT0[:, :, 16 * g : 16 * (g + 1)]
                srcv = sg[:].rearrange("p c x -> p x c")
                if g % 2 == 0:
                    nc.scalar.copy(dst, srcv)
                else:
                    nc.vector.tensor_copy(dst, srcv)

            # ---- main loop: L0 stream with L1/L2 A-form scribbles woven in
            for t in range(8):
                i0 = t * 2
                st = scrib.tile([128, 2, 1024], f32, tag="st0")
                nc.gpsimd.dma_start(
                    out=st[:],
                    in_=scr[i0 : i0 + 2]
                    .rearrange("i (y k) x -> y i k x", k=4)[:, :, 1:3, :]
                    .rearrange("y i k x -> y i (k x)"),
                )
                for il in range(2):
                    feat_load_copy(2 * t + il)
                    mask_ops(
                        0, msk0[:, i0 + il, :],
                        st[:, il, 0:512], st[:, il, 512:1024], 128, 128,
                    )
                # L1 A-form tile t (= ilo): partitions q = 32*ihi + g
                st1 = scrib.tile([64, 2, 2, 512], f32, tag="st1")
                for ylo in range(2):
                    nc.gpsimd.dma_start(
                        out=st1[:, ylo, :, :],
                        in_=scr.rearrange(
                            "(ihi ilo) (g r) x -> ilo ihi g r x", ihi=2, r=16
                        )[t, :, :, 8 * ylo + 3 : 8 * ylo + 5, :],
                    )
                    mask_ops(
                        1, msk1[:, t, ylo, :],
                        st1[:, ylo, 0, :], st1[:, ylo, 1, :], 64, 64,
                    )
                # L2 A-form tile t//2 (= ilo), two yq per iteration
                ilo2, yq0 = t // 2, (t % 2) * 2
                st2 = scrib.tile([32, 2, 2, 512], f32, tag="st2")
                for j in range(2):
                    yq = yq0 + j
                    nc.gpsimd.dma_start(
                        out=st2[:, j, :, :],
                        in_=scr.rearrange(
                            "(ihi ilo) (g r) x -> ilo ihi g r x", ihi=4, r=64
                        )[ilo2, :, :, 16 * yq + 7 : 16 * yq + 9, :],
                    )
                    mask_ops(
                        2, msk2[:, ilo2, yq, :],
                        st2[:, j, 0, :], st2[:, j, 1, :], 32, 32,
                    )

            # ---- L1/L2 mask PE transposes -> mT tiles (drains on DVE)
            for ilo in range(8):
                psM = psum.tile([128, 64], bf16, tag="psM")
                nc.tensor.transpose(
                    psM[:],
                    msk1[:, ilo, :, :].rearrange("p a b -> p (a b)"),
                    ident[0:64, 0:64],
                )
                nc.vector.tensor_copy(
                    mT1[:, 2 * ilo : 2 * ilo + 2, :],
                    psM[:].rearrange("p (h g) -> p h g", h=2),
                )
            for ilo in range(4):
                psM = psum.tile([128, 32], bf16, tag="psM")
                nc.tensor.transpose(
                    psM[:],
                    msk2[:, ilo, :, :].rearrange("p a b -> p (a b)"),
                    ident[0:32, 0:32],
                )
                nc.vector.tensor_copy(
                    mT2[:, 4 * ilo : 4 * ilo + 4, :],
                    psM[:].rearrange("p (h g) -> p h g", h=4),
                )

            # ---- L1/L2 matmuls (emitted before L0's: they only need data
            # that lands early, so they hide inside L0's DMA window)
            for li, T, mT, G in ((1, T1, mT1, 32), (2, T2, mT2, 8)):
                acc = psum.tile([_I, _C], f32, tag="acc")
                for g_ in range(G):
                    nc.tensor.matmul(
                        acc[:], mT[:, :, g_], T[:, g_, :, :],
                        start=(g_ == 0), stop=(g_ == G - 1),
                    )
                r = singles.tile([128, _I], f32, tag=f"r{li}")
                nc.vector.reduce_sum(out=r[:], in_=mT[:], axis=X)
                cnt = psum.tile([_I, 1], f32, tag="cnt")
                nc.tensor.matmul(cnt[:], r[:], ones[:], start=True, stop=True)
                base = li * (_C + 1)
                nc.vector.tensor_copy(stag[:, base : base + _C], acc[:])
                nc.vector.tensor_copy(stag[:, base + _C : base + _C + 1], cnt[:])

            # ---- L0 matmuls last (the kernel tail)
            acc = psum.tile([_I, _C], f32, tag="acc")
            for x in range(128):
                nc.tensor.matmul(
                    acc[:], msk0[:, :, x], sgT0[:, x, :],
                    start=(x == 0), stop=(x == 127),
                )
            r0 = singles.tile([128, _I], f32, tag="r0")
            nc.vector.reduce_sum(out=r0[:], in_=msk0[:], axis=X)
            cnt0 = psum.tile([_I, 1], f32, tag="cnt")
            nc.tensor.matmul(cnt0[:], r0[:], ones[:], start=True, stop=True)
            nc.vector.tensor_copy(stag[:, 0:_C], acc[:])
            nc.vector.tensor_copy(stag[:, _C : _C + 1], cnt0[:])

            nc.gpsimd.dma_start(out=out_d[:], in_=stag[:])

    nc.compile()
    return nc


def _host_fallback(scr_bi, fmap_b, h, k, off):
    """Feature at argmax of the soft mask; only used when a mask is empty."""
    V = scr_bi[off::k, :][:h].astype(np.float32) + scr_bi[off + 1 :: k, :][:h]
    sr4 = V[:, off::k][:, :h] + V[:, off + 1 :: k][:, :h]
    idx = int(np.argmax(np.float32(0.25) * sr4))
    y, x = divmod(idx, h)
    return fmap_b[:, y, x]


def kernel(feat0, feat1, feat2, scribbles):
    import sys

    for p in ("/opt/trn_rl_repo", "/opt/pypackages"):
        if p not in sys.path:
            sys.path.append(p)
    from concourse.bass_utils import run_bass_kernel_spmd
    import ml_dtypes

    feat0 = np.asarray(feat0, dtype=np.float32)
    feat1 = np.asarray(feat1, dtype=np.float32)
    feat2 = np.asarray(feat2, dtype=np.float32)
    scribbles = np.asarray(scribbles, dtype=np.float32)
    ident = np.eye(128, dtype=ml_dtypes.bfloat16)

    nc = _build_nc()
    in_maps = [
        {
            "feat0": np.ascontiguousarray(feat0[b]),
            "feat1": np.ascontiguousarray(feat1[b]),
            "feat2": np.ascontiguousarray(feat2[b]),
            "scribbles": np.ascontiguousarray(scribbles[b]),
            "ident": ident,
        }
        for b in range(_B)
    ]
    res = run_bass_kernel_spmd(nc, in_maps, core_ids=list(range(_B)))
    raw = np.stack([res.results[b]["out"] for b in range(_B)])  # [B, I, 3*257]
    raw = raw.reshape(_B, _I, 3, _C + 1)
    # un-permute the transpose-induced image order: row _IMG2POS[l][i] = image i
    for li in (1, 2):
        raw[:, :, li, :] = raw[:, _IMG2POS[li], li, :]
    ssum = raw[..., :_C].astype(np.float32)  # [B, I, 3, C]
    cnt = raw[..., _C].astype(np.float32)  # [B, I, 3]

    mean = ssum / np.maximum(cnt, np.float32(1.0))[..., None]

    if (cnt == 0).any():  # never for non-degenerate inputs
        fm = [feat0, feat1, feat2]
        for b, i, li in zip(*np.nonzero(cnt == 0)):
            h, k, off = _LEVELS[li]
            mean[b, i, li] = _host_fallback(scribbles[b, i], fm[li][b], h, k, off)

    out = (mean[:, :, 0] + mean[:, :, 1] + mean[:, :, 2]) / np.float32(3.0)
    return out.astype(np.float32)


# revision 34
# speedup vs baseline: 1.0214x; 1.0214x over previous
"""Trainium2 Bass kernel: multi-scale masked average-pool descriptors.

Computes, per batch element b and scribble i:
    d_l[b,i,c] = mean over {pixels where resize(scribble)[b,i,y,x] > 0.5} of feat_l[b,c,y,x]
    out[b,i,c] = (d_0 + d_1 + d_2) / 3

Strategy (v5 -- all constants below are HW-measured):
  * jax.image.resize(bilinear, antialias=False) at scales 4/8/16 reduces to an
    exact 2x2 average at stride k with offset o (k,o) = (4,1)/(8,3)/(16,7):
    mask == ((a+c)+(b+d)) > 2.0 bit-exactly in fp32 (computed on DVE, only
    the 2-of-k needed columns).  Scribbles ride the gpsimd SWDGE queue as
    merged 4KB row-pair descriptors.
  * Level 0 features (16.8MB, the bulk) are DMA'd with FULL-ROW descriptors
    ([y, 16c, 128x] tiles, one 512B descriptor per (c,y) row) split across
    the two HWDGE rings -- the descriptor walk performs the [c,y,x]->[y,..]
    partition transpose for free at ~233 GB/s.  Engine copies (cross-assigned
    scalar/DVE) re-pack each group into one assembled [y, x, 256c] bf16 tile
    so each of the 128 x-column matmuls has a contiguous [128, 256] rhs
    (251ns/LDW+MM pair measured; strided rhs would be 3x slower).  These
    matmuls are emitted LAST: they are the only consumer of the final DMA
    bytes and form the ~20us kernel tail.
  * Levels 1/2 features load in native [c, s] layout (contiguous 8-16KB
    descriptors, bf16-cast inline by the gpsimd SWDGE), are transposed by
    the PE (128x128 identity matmuls into PSUM, batched 4 blocks/bank) into
    raster [s, c] tiles, and run one matmul per 128-pixel raster group --
    their whole pipeline hides inside level 0's DMA window.
  * L1/L2 masks are built in a partition fold (q=32*ihi+g / q=8*ihi+g) such
    that a PE transpose of each ilo-slice directly yields the matmul lhsT
    [128px, 16img] (image order comes out permuted; the host un-permutes).
  * cnt[i] = reduce_sum over the mask + a ones-matmul; bf16 masks are exact
    0/1 and PSUM accumulates fp32, so cnt is exact and masks match the
    reference bit-exactly.  bf16 features give rel err ~2e-3 (gate: 2e-2).
  * The empty-mask fallback is handled on the host (P(empty) ~ 2^-1024).

Sharding: pure data-parallel over batch B=8 across the 8 NeuronCores.
"""

import numpy as np

_B = 8
_I = 16
_C = 256

# level: (h, k, off)
_LEVELS = {0: (128, 4, 1), 1: (64, 8, 3), 2: (32, 16, 7)}

# PE-transpose-induced image permutation (levels 1/2): matmul M position of
# image i.  i = ihi*(16/hh) + ilo  ->  b = ilo*hh + ihi  (hh = 2 for L1, 4
# for L2).
_IMG2POS = {
    1: [(i % 8) * 2 + i // 8 for i in range(16)],
    2: [(i % 4) * 4 + i // 4 for i in range(16)],
}


def _build_nc():
    import concourse.bacc as bacc
    import concourse.tile as tile
    from concourse import mybir

    f32 = mybir.dt.float32
    bf16 = mybir.dt.bfloat16
    gt = mybir.AluOpType.is_gt
    X = mybir.AxisListType.X

    nc = bacc.Bacc("TRN2", target_bir_lowering=False, debug=False)

    feats = {
        0: nc.dram_tensor("feat0", [_C, 128, 128], f32, kind="ExternalInput"),
        1: nc.dram_tensor("feat1", [_C, 64, 64], f32, kind="ExternalInput"),
        2: nc.dram_tensor("feat2", [_C, 32, 32], f32, kind="ExternalInput"),
    }
    scr = nc.dram_tensor("scribbles", [_I, 512, 512], f32, kind="ExternalInput")
    ident_d = nc.dram_tensor("ident", [128, 128], bf16, kind="ExternalInput")
    out_d = nc.dram_tensor("out", [_I, 3 * (_C + 1)], f32, kind="ExternalOutput")

    with tile.TileContext(nc) as tc:
        with (
            tc.tile_pool(name="singles", bufs=1) as singles,
            tc.tile_pool(name="scrib", bufs=2) as scrib,
            tc.tile_pool(name="tmp", bufs=2) as tmp,
            tc.tile_pool(name="fR", bufs=4) as fR,
            tc.tile_pool(name="fc", bufs=1) as fc,
            tc.tile_pool(name="psum", bufs=2, space="PSUM") as psum,
        ):
            ones = singles.tile([128, 1], f32, tag="ones")
            nc.vector.memset(ones[:], 1.0)
            stag = singles.tile([_I, 3 * (_C + 1)], f32, tag="stag")
            ident = singles.tile([128, 128], bf16, tag="ident")
            nc.sync.dma_start(out=ident[:], in_=ident_d[:])

            # masks. L0: y-layout [y, i, x].  L1/L2: A-form fold
            # msk_l[q, ilo, ysub, x] with q = (32|8)*ihi + g, y = hh*g + ysub
            msk0 = singles.tile([128, _I, 128], bf16, tag="msk0")
            msk1 = singles.tile([64, 8, 2, 64], bf16, tag="msk1")
            msk2 = singles.tile([32, 4, 4, 32], bf16, tag="msk2")
            # transposed lhsT tiles [128px, 16b, G]
            mT1 = singles.tile([128, _I, 32], bf16, tag="mT1")
            mT2 = singles.tile([128, _I, 8], bf16, tag="mT2")
            # feature tiles: L0 assembled [y, x, c]; L1/L2 raster [s, g, H, c]
            sgT0 = singles.tile([128, 128, _C], bf16, tag="sgT0")
            T1 = singles.tile([128, 32, 2, 128], bf16, tag="T1")
            T2 = singles.tile([128, 8, 2, 128], bf16, tag="T2")

            # ---- L1/L2 feature cast-loads (gpsimd, contiguous descriptors)
            sgc = {}
            for li, S in ((1, 4096), (2, 1024)):
                for H in range(2):
                    sg = fc.tile([128, S], bf16, tag=f"sgc{li}{H}")
                    nc.gpsimd.dma_start(
                        out=sg[:],
                        in_=feats[li].rearrange("c y x -> c (y x)")[
                            128 * H : 128 * (H + 1), :
                        ],
                    )
                    sgc[(li, H)] = sg

            # ---- L1/L2 feature PE transposes, batched 4 blocks per PSUM
            # bank.  The PSUM->SBUF drains are deferred: they run on the DVE
            # woven into the main loop so they never block its FIFO head.
            drains = []
            for li, T in ((1, T1), (2, T2)):
                S = 4096 if li == 1 else 1024
                for H in range(2):
                    sg = sgc[(li, H)]
                    for q in range(S // 512):
                        psB = psum.tile([128, 512], bf16, tag="psB")
                        for b_ in range(4):
                            blk = 4 * q + b_
                            nc.tensor.transpose(
                                psB[:, 128 * b_ : 128 * (b_ + 1)],
                                sg[:, 128 * blk : 128 * (blk + 1)],
                                ident[:],
                            )
                        drains.append(
                            (T[:, 4 * q : 4 * q + 4, H, :],
                             psB[:].rearrange("p (g c) -> p g c", g=4))
                        )

            def drain_some(n):
                for _ in range(n):
                    if drains:
                        dstT, srcB = drains.pop(0)
                        nc.vector.tensor_copy(dstT, srcB)

            # ---- mask builders ------------------------------------------
            def mask_ops(li, out_ap, src_lo, src_hi, h, w):
                k, off = _LEVELS[li][1], _LEVELS[li][2]
                a = src_lo.rearrange("p (x k) -> p x k", k=k)[:, :, off : off + 2]
                b = src_hi.rearrange("p (x k) -> p x k", k=k)[:, :, off : off + 2]
                v = tmp.tile([h, w, 2], f32, tag=f"v{li}")
                nc.vector.tensor_add(v[:], a, b)
                sr = tmp.tile([h, w], f32, tag=f"sr{li}")
                nc.vector.tensor_add(sr[:], v[:, :, 0], v[:, :, 1])
                nc.vector.tensor_scalar(
                    out=out_ap, in0=sr[:], scalar1=2.0, scalar2=None, op0=gt
                )

            def feat_load_copy(g):
                # L0: 16-channel full-row staging groups; loads alternate the
                # two HWDGE rings (descriptor gen is ~1.4ns/desc); copies are
                # cross-assigned (scalar copies what sync loaded, DVE copies
                # what scalar loaded) so a copy never waits on its own
                # engine's queue head.
                sg = fR.tile([128, 16, 128], f32, tag="sgR")
                deng = nc.sync if g % 2 == 0 else nc.scalar
                deng.dma_start(
                    out=sg[:],
                    in_=feats[0][16 * g : 16 * (g + 1)].rearrange(
                        "c y x -> y c x"
                    ),
                )
                nc.scalar.copy(
                    sgT0[:, :, 16 * g : 16 * (g + 1)],
                    sg[:].rearrange("p c x -> p x c"),
                )

            # ---- main loop: L0 stream with L1/L2 A-form scribbles woven in
            for t in range(8):
                i0 = t * 2
                st = scrib.tile([128, 2, 1024], f32, tag="st0")
                nc.gpsimd.dma_start(
                    out=st[:],
                    in_=scr[i0 : i0 + 2]
                    .rearrange("i (y k) x -> y i k x", k=4)[:, :, 1:3, :]
                    .rearrange("y i k x -> y i (k x)"),
                )
                for il in range(2):
                    feat_load_copy(2 * t + il)
                    mask_ops(
                        0, msk0[:, i0 + il, :],
                        st[:, il, 0:512], st[:, il, 512:1024], 128, 128,
                    )
                # L1 A-form tile t (= ilo): partitions q = 32*ihi + g
                st1 = scrib.tile([64, 2, 2, 512], f32, tag="st1")
                for ylo in range(2):
                    nc.gpsimd.dma_start(
                        out=st1[:, ylo, :, :],
                        in_=scr.rearrange(
                            "(ihi ilo) (g r) x -> ilo ihi g r x", ihi=2, r=16
                        )[t, :, :, 8 * ylo + 3 : 8 * ylo + 5, :],
                    )
                    mask_ops(
                        1, msk1[:, t, ylo, :],
                        st1[:, ylo, 0, :], st1[:, ylo, 1, :], 64, 64,
                    )
                # L2 A-form tile t//2 (= ilo), two yq per iteration
                ilo2, yq0 = t // 2, (t % 2) * 2
                st2 = scrib.tile([32, 2, 2, 512], f32, tag="st2")
                for j in range(2):
                    yq = yq0 + j
                    nc.gpsimd.dma_start(
                        out=st2[:, j, :, :],
                        in_=scr.rearrange(
                            "(ihi ilo) (g r) x -> ilo ihi g r x", ihi=4, r=64
                        )[ilo2, :, :, 16 * yq + 7 : 16 * yq + 9, :],
                    )
                    mask_ops(
                        2, msk2[:, ilo2, yq, :],
                        st2[:, j, 0, :], st2[:, j, 1, :], 32, 32,
                    )
                drain_some(3)

            drain_some(len(drains))

            # ---- L1/L2 mask PE transposes -> mT tiles (drains on DVE)
            for ilo in range(8):
                psM = psum.tile([128, 64], bf16, tag="psM")
                nc.tensor.transpose(
                    psM[:],
                    msk1[:, ilo, :, :].rearrange("p a b -> p (a b)"),
                    ident[0:64, 0:64],
                )
                nc.vector.tensor_copy(
                    mT1[:, 2 * ilo : 2 * ilo + 2, :],
                    psM[:].rearrange("p (h g) -> p h g", h=2),
                )
            for ilo in range(4):
                psM = psum.tile([128, 32], bf16, tag="psM")
                nc.tensor.transpose(
                    psM[:],
                    msk2[:, ilo, :, :].rearrange("p a b -> p (a b)"),
                    ident[0:32, 0:32],
                )
                nc.vector.tensor_copy(
                    mT2[:, 4 * ilo : 4 * ilo + 4, :],
                    psM[:].rearrange("p (h g) -> p h g", h=4),
                )

            # ---- L1/L2 matmuls (emitted before L0's: they only need data
            # that lands early, so they hide inside L0's DMA window)
            for li, T, mT, G in ((1, T1, mT1, 32), (2, T2, mT2, 8)):
                acc = psum.tile([_I, _C], f32, tag="acc")
                for g_ in range(G):
                    nc.tensor.matmul(
                        acc[:], mT[:, :, g_], T[:, g_, :, :],
                        start=(g_ == 0), stop=(g_ == G - 1),
                    )
                r = singles.tile([128, _I], f32, tag=f"r{li}")
                nc.vector.reduce_sum(out=r[:], in_=mT[:], axis=X)
                cnt = psum.tile([_I, 1], f32, tag="cnt")
                nc.tensor.matmul(cnt[:], r[:], ones[:], start=True, stop=True)
                base = li * (_C + 1)
                nc.vector.tensor_copy(stag[:, base : base + _C], acc[:])
                nc.vector.tensor_copy(stag[:, base + _C : base + _C + 1], cnt[:])

            # ---- L0 matmuls last (the kernel tail)
            acc = psum.tile([_I, _C], f32, tag="acc")
            for x in range(128):
                nc.tensor.matmul(
                    acc[:], msk0[:, :, x], sgT0[:, x, :],
                    start=(x == 0), stop=(x == 127),
                )
            r0 = singles.tile([128, _I], f32, tag="r0")
            nc.vector.reduce_sum(out=r0[:], in_=msk0[:], axis=X)
            cnt0 = psum.tile([_I, 1], f32, tag="cnt")
            nc.tensor.matmul(cnt0[:], r0[:], ones[:], start=True, stop=True)
            nc.vector.tensor_copy(stag[:, 0:_C], acc[:])
            nc.vector.tensor_copy(stag[:, _C : _C + 1], cnt0[:])

            nc.gpsimd.dma_start(out=out_d[:], in_=stag[:])

    nc.compile()
    return nc


def _host_fallback(scr_bi, fmap_b, h, k, off):
    """Feature at argmax of the soft mask; only used when a mask is empty."""
    V = scr_bi[off::k, :][:h].astype(np.float32) + scr_bi[off + 1 :: k, :][:h]
    sr4 = V[:, off::k][:, :h] + V[:, off + 1 :: k][:, :h]
    idx = int(np.argmax(np.float32(0.25) * sr4))
    y, x = divmod(idx, h)
    return fmap_b[:, y, x]


def kernel(feat0, feat1, feat2, scribbles):
    import sys

    for p in ("/opt/trn_rl_repo", "/opt/pypackages"):
        if p not in sys.path:
            sys.path.append(p)
    from concourse.bass_utils import run_bass_kernel_spmd
    import ml_dtypes

    feat0 = np.asarray(feat0, dtype=np.float32)
    feat1 = np.asarray(feat1, dtype=np.float32)
    feat2 = np.asarray(feat2, dtype=np.float32)
    scribbles = np.asarray(scribbles, dtype=np.float32)
    ident = np.eye(128, dtype=ml_dtypes.bfloat16)

    nc = _build_nc()
    in_maps = [
        {
            "feat0": np.ascontiguousarray(feat0[b]),
            "feat1": np.ascontiguousarray(feat1[b]),
            "feat2": np.ascontiguousarray(feat2[b]),
            "scribbles": np.ascontiguousarray(scribbles[b]),
            "ident": ident,
        }
        for b in range(_B)
    ]
    res = run_bass_kernel_spmd(nc, in_maps, core_ids=list(range(_B)))
    raw = np.stack([res.results[b]["out"] for b in range(_B)])  # [B, I, 3*257]
    raw = raw.reshape(_B, _I, 3, _C + 1)
    # un-permute the transpose-induced image order: row _IMG2POS[l][i] = image i
    for li in (1, 2):
        raw[:, :, li, :] = raw[:, _IMG2POS[li], li, :]
    ssum = raw[..., :_C].astype(np.float32)  # [B, I, 3, C]
    cnt = raw[..., _C].astype(np.float32)  # [B, I, 3]

    mean = ssum / np.maximum(cnt, np.float32(1.0))[..., None]

    if (cnt == 0).any():  # never for non-degenerate inputs
        fm = [feat0, feat1, feat2]
        for b, i, li in zip(*np.nonzero(cnt == 0)):
            h, k, off = _LEVELS[li]
            mean[b, i, li] = _host_fallback(scribbles[b, i], fm[li][b], h, k, off)

    out = (mean[:, :, 0] + mean[:, :, 1] + mean[:, :, 2]) / np.float32(3.0)
    return out.astype(np.float32)


# revision 35
# speedup vs baseline: 1.1413x; 1.1174x over previous
"""Trainium2 Bass kernel: multi-scale masked average-pool descriptors.

Computes, per batch element b and scribble i:
    d_l[b,i,c] = mean over {pixels where resize(scribble)[b,i,y,x] > 0.5} of feat_l[b,c,y,x]
    out[b,i,c] = (d_0 + d_1 + d_2) / 3

Strategy (v4 -- all-measured design):
  * jax.image.resize(bilinear, antialias=False) at scales 4/8/16 reduces to an
    exact 2x2 average at stride k with offset o (k,o) = (4,1)/(8,3)/(16,7):
    mask == ((a+c)+(b+d)) > 2.0 bit-exactly in fp32 (computed on DVE).
    Scribbles ride the gpsimd SWDGE queue as merged 4KB row-pair descriptors.
  * Feature maps are DMA'd with FULL-ROW descriptors ([y, c-group, x] tiles,
    one 512/256/128B descriptor per (c,y) row) on the two HWDGE rings -- the
    DMA descriptor walk performs the [c,y,x] -> [y,...] partition transpose
    for free (~233 GB/s measured; the xbar and PE transpose alternatives
    measured slower and/or serialize against all other DMA).
  * Engine copies re-pack each c-group [y, 32c, w] fp32 into assembled
    [y, x, 256c] bf16 tiles (cast during copy), so every matmul rhs is a
    contiguous [h, 256] bf16 slice.
  * ssum[i,:] accumulates as one matmul per pixel column x: lhsT =
    mask[:, :, x] [h, 16] bf16, rhs = f[:, x, :] [h, 256] bf16 -- measured
    251ns per LDWEIGHTS+MATMUL pair (strided rhs would be 779ns).
  * cnt[i] = reduce_sum over the mask + a ones-matmul; bf16 masks are exact
    0/1 and PSUM accumulates fp32, so cnt is exact and masks match the
    reference bit-exactly.  bf16 features give rel err ~2e-3 (gate: 2e-2).
  * The empty-mask fallback is handled on the host (P(empty) ~ 2^-1024).

Sharding: pure data-parallel over batch B=8 across the 8 NeuronCores.
"""

import numpy as np

_B = 8
_I = 16
_C = 256

# level: (h, k, off)
_LEVELS = {0: (128, 4, 1), 1: (64, 8, 3), 2: (32, 16, 7)}


def _build_nc():
    import concourse.bacc as bacc
    import concourse.tile as tile
    from concourse import mybir

    f32 = mybir.dt.float32
    bf16 = mybir.dt.bfloat16
    gt = mybir.AluOpType.is_gt
    X = mybir.AxisListType.X

    nc = bacc.Bacc("TRN2", target_bir_lowering=False, debug=False)

    feats = {
        0: nc.dram_tensor("feat0", [_C, 128, 128], f32, kind="ExternalInput"),
        1: nc.dram_tensor("feat1", [_C, 64, 64], f32, kind="ExternalInput"),
        2: nc.dram_tensor("feat2", [_C, 32, 32], f32, kind="ExternalInput"),
    }
    scr = nc.dram_tensor("scribbles", [_I, 512, 512], f32, kind="ExternalInput")
    out_d = nc.dram_tensor("out", [_I, 3 * (_C + 1)], f32, kind="ExternalOutput")

    with tile.TileContext(nc) as tc:
        with (
            tc.tile_pool(name="singles", bufs=1) as singles,
            tc.tile_pool(name="scrib", bufs=2) as scrib,
            tc.tile_pool(name="scrib2", bufs=2) as scrib2,
            tc.tile_pool(name="tmp", bufs=2) as tmp,
            tc.tile_pool(name="fR", bufs=4) as fR,
            tc.tile_pool(name="psum", bufs=3, space="PSUM") as psum,
        ):
            ones = singles.tile([128, 1], f32, tag="ones")
            nc.vector.memset(ones[:], 1.0)
            stag = singles.tile([_I, 3 * (_C + 1)], f32, tag="stag")

            # masks, y-on-partitions (natural resize layout): msk_l[y, i, x]
            msk0 = singles.tile([128, _I, 128], bf16, tag="msk0")
            msk1 = singles.tile([64, _I, 64], bf16, tag="msk1")
            msk2 = singles.tile([32, _I, 32], bf16, tag="msk2")
            msk = {0: msk0, 1: msk1, 2: msk2}
            # assembled feature tiles [y, x, c] bf16
            sgT0 = singles.tile([128, 128, _C], bf16, tag="sgT0")
            sgT1 = singles.tile([64, 64, _C], bf16, tag="sgT1")
            sgT2 = singles.tile([32, 32, _C], bf16, tag="sgT2")
            sgT = {0: sgT0, 1: sgT1, 2: sgT2}

            # ---- interleaved per-level streams ----------------------
            # Queues: gpsimd = scribbles (4KB row-pair descs), sync/scalar =
            # feature full-row loads.  The DVE FIFO alternates one feature
            # assembly copy with one tile's mask ALU so neither stream
            # stalls the other; emission order == engine FIFO order.

            def mask_ops(li, i, st, il=None):
                # only the 2-of-k needed resize columns are added (strided)
                h, k, off = _LEVELS[li]
                src_lo = st[:, il, 0:512] if il is not None else st[:, 0, :]
                src_hi = st[:, il, 512:1024] if il is not None else st[:, 1, :]
                a = src_lo.rearrange("p (x k) -> p x k", k=k)[:, :, off : off + 2]
                b = src_hi.rearrange("p (x k) -> p x k", k=k)[:, :, off : off + 2]
                v = tmp.tile([h, h, 2], f32, tag="v")
                nc.vector.tensor_add(v[:], a, b)
                sr = tmp.tile([h, h], f32, tag="sr")
                nc.vector.tensor_add(sr[:], v[:, :, 0], v[:, :, 1])
                nc.vector.tensor_scalar(
                    out=msk[li][:, i, :], in0=sr[:], scalar1=2.0,
                    scalar2=None, op0=gt,
                )

            def feat_load_copy(li, g):
                # 16-channel staging groups; loads alternate the two HWDGE
                # rings (descriptor generation is ~1.4ns/desc and must be
                # split).  Copies are cross-assigned -- scalar copies what
                # sync loaded and the DVE copies what scalar loaded -- so a
                # copy never waits on its own engine's queue head.
                h = _LEVELS[li][0]
                sg = fR.tile([h, 16, h], f32, tag="sgR")
                deng = nc.sync if g % 2 == 0 else nc.scalar
                deng.dma_start(
                    out=sg[:],
                    in_=feats[li][16 * g : 16 * (g + 1)].rearrange(
                        "c y x -> y c x"
                    ),
                )
                dst = sgT[li][:, :, 16 * g : 16 * (g + 1)]
                srcv = sg[:].rearrange("p c x -> p x c")
                if g % 2 == 0:
                    nc.scalar.copy(dst, srcv)
                else:
                    nc.vector.tensor_copy(dst, srcv)

            def scrib_load(li, i):
                h, k, off = _LEVELS[li]
                rr = 512 // h
                st = scrib2.tile([h, 2, 512], f32, tag=f"st{li}")
                nc.gpsimd.dma_start(
                    out=st[:],
                    in_=scr[i].rearrange("(y r) x -> y r x", r=rr)[
                        :, off : off + 2, :
                    ],
                )
                return st

            # main loop: L0 features+masks with the L1/L2 scribble+mask
            # streams interleaved round-robin so they finish inside L0's
            # DMA phase instead of trailing it
            for t in range(8):
                i0 = t * 2
                st = scrib.tile([128, 2, 1024], f32, tag="st0")
                nc.gpsimd.dma_start(
                    out=st[:],
                    in_=scr[i0 : i0 + 2]
                    .rearrange("i (y k) x -> y i k x", k=4)[:, :, 1:3, :]
                    .rearrange("y i k x -> y i (k x)"),
                )
                for il in range(2):
                    i = i0 + il
                    feat_load_copy(0, 2 * t + il)
                    mask_ops(0, i, st, il=il)
                    mask_ops(1, i, scrib_load(1, i))
                    mask_ops(2, i, scrib_load(2, i))

            for li in (1, 2):
                for i in range(_I):
                    feat_load_copy(li, i)

            # ---- matmuls + cnt + staging, level order 0, 1, 2
            for li in (0, 1, 2):
                h = _LEVELS[li][0]
                acc = psum.tile([_I, _C], f32, tag="acc")
                for x in range(h):
                    nc.tensor.matmul(
                        acc[:], msk[li][:, :, x], sgT[li][:, x, :],
                        start=(x == 0), stop=(x == h - 1),
                    )
                r = singles.tile([h, _I], f32, tag=f"r{li}")
                nc.vector.reduce_sum(out=r[:], in_=msk[li][:], axis=X)
                cnt = psum.tile([_I, 1], f32, tag="cnt")
                nc.tensor.matmul(cnt[:], r[:], ones[:h, :], start=True, stop=True)
                base = li * (_C + 1)
                nc.vector.tensor_copy(stag[:, base : base + _C], acc[:])
                nc.vector.tensor_copy(stag[:, base + _C : base + _C + 1], cnt[:])

            nc.gpsimd.dma_start(out=out_d[:], in_=stag[:])

    nc.compile()
    return nc


def _host_fallback(scr_bi, fmap_b, h, k, off):
    """Feature at argmax of the soft mask; only used when a mask is empty."""
    V = scr_bi[off::k, :][:h].astype(np.float32) + scr_bi[off + 1 :: k, :][:h]
    sr4 = V[:, off::k][:, :h] + V[:, off + 1 :: k][:, :h]
    idx = int(np.argmax(np.float32(0.25) * sr4))
    y, x = divmod(idx, h)
    return fmap_b[:, y, x]


def kernel(feat0, feat1, feat2, scribbles):
    import sys

    for p in ("/opt/trn_rl_repo", "/opt/pypackages"):
        if p not in sys.path:
            sys.path.append(p)
    from concourse.bass_utils import run_bass_kernel_spmd

    feat0 = np.asarray(feat0, dtype=np.float32)
    feat1 = np.asarray(feat1, dtype=np.float32)
    feat2 = np.asarray(feat2, dtype=np.float32)
    scribbles = np.asarray(scribbles, dtype=np.float32)

    nc = _build_nc()
    in_maps = [
        {
            "feat0": np.ascontiguousarray(feat0[b]),
            "feat1": np.ascontiguousarray(feat1[b]),
            "feat2": np.ascontiguousarray(feat2[b]),
            "scribbles": np.ascontiguousarray(scribbles[b]),
        }
        for b in range(_B)
    ]
    res = run_bass_kernel_spmd(nc, in_maps, core_ids=list(range(_B)))
    raw = np.stack([res.results[b]["out"] for b in range(_B)])  # [B, I, 3*257]
    raw = raw.reshape(_B, _I, 3, _C + 1)
    ssum = raw[..., :_C].astype(np.float32)  # [B, I, 3, C]
    cnt = raw[..., _C].astype(np.float32)  # [B, I, 3]

    mean = ssum / np.maximum(cnt, np.float32(1.0))[..., None]

    if (cnt == 0).any():  # never for non-degenerate inputs
        fm = [feat0, feat1, feat2]
        for b, i, li in zip(*np.nonzero(cnt == 0)):
            h, k, off = _LEVELS[li]
            mean[b, i, li] = _host_fallback(scribbles[b, i], fm[li][b], h, k, off)

    out = (mean[:, :, 0] + mean[:, :, 1] + mean[:, :, 2]) / np.float32(3.0)
    return out.astype(np.float32)


# revision 36
# speedup vs baseline: 1.1756x; 1.0300x over previous
"""Trainium2 Bass kernel: multi-scale masked average-pool descriptors.

Computes, per batch element b and scribble i:
    d_l[b,i,c] = mean over {pixels where resize(scribble)[b,i,y,x] > 0.5} of feat_l[b,c,y,x]
    out[b,i,c] = (d_0 + d_1 + d_2) / 3

Strategy (v4 -- all-measured design):
  * jax.image.resize(bilinear, antialias=False) at scales 4/8/16 reduces to an
    exact 2x2 average at stride k with offset o (k,o) = (4,1)/(8,3)/(16,7):
    mask == ((a+c)+(b+d)) > 2.0 bit-exactly in fp32 (computed on DVE).
    Scribbles ride the gpsimd SWDGE queue as merged 4KB row-pair descriptors.
  * Feature maps are DMA'd with FULL-ROW descriptors ([y, c-group, x] tiles,
    one 512/256/128B descriptor per (c,y) row) on the two HWDGE rings -- the
    DMA descriptor walk performs the [c,y,x] -> [y,...] partition transpose
    for free (~233 GB/s measured; the xbar and PE transpose alternatives
    measured slower and/or serialize against all other DMA).
  * Engine copies re-pack each c-group [y, 32c, w] fp32 into assembled
    [y, x, 256c] bf16 tiles (cast during copy), so every matmul rhs is a
    contiguous [h, 256] bf16 slice.
  * ssum[i,:] accumulates as one matmul per pixel column x: lhsT =
    mask[:, :, x] [h, 16] bf16, rhs = f[:, x, :] [h, 256] bf16 -- measured
    251ns per LDWEIGHTS+MATMUL pair (strided rhs would be 779ns).
  * cnt[i] = reduce_sum over the mask + a ones-matmul; bf16 masks are exact
    0/1 and PSUM accumulates fp32, so cnt is exact and masks match the
    reference bit-exactly.  bf16 features give rel err ~2e-3 (gate: 2e-2).
  * The empty-mask fallback is handled on the host (P(empty) ~ 2^-1024).

Sharding: pure data-parallel over batch B=8 across the 8 NeuronCores.
"""

import numpy as np

_B = 8
_I = 16
_C = 256

# level: (h, k, off)
_LEVELS = {0: (128, 4, 1), 1: (64, 8, 3), 2: (32, 16, 7)}


def _build_nc():
    import concourse.bacc as bacc
    import concourse.tile as tile
    from concourse import mybir

    f32 = mybir.dt.float32
    bf16 = mybir.dt.bfloat16
    gt = mybir.AluOpType.is_gt
    X = mybir.AxisListType.X

    nc = bacc.Bacc("TRN2", target_bir_lowering=False, debug=False)

    feats = {
        0: nc.dram_tensor("feat0", [_C, 128, 128], f32, kind="ExternalInput"),
        1: nc.dram_tensor("feat1", [_C, 64, 64], f32, kind="ExternalInput"),
        2: nc.dram_tensor("feat2", [_C, 32, 32], f32, kind="ExternalInput"),
    }
    scr = nc.dram_tensor("scribbles", [_I, 512, 512], f32, kind="ExternalInput")
    out_d = nc.dram_tensor("out", [_I, 3 * (_C + 1)], f32, kind="ExternalOutput")

    with tile.TileContext(nc) as tc:
        with (
            tc.tile_pool(name="singles", bufs=1) as singles,
            tc.tile_pool(name="scrib", bufs=2) as scrib,
            tc.tile_pool(name="scrib2", bufs=3) as scrib2,
            tc.tile_pool(name="tmp", bufs=2) as tmp,
            tc.tile_pool(name="fR", bufs=4) as fR,
            tc.tile_pool(name="psum", bufs=3, space="PSUM") as psum,
        ):
            ones = singles.tile([128, 1], f32, tag="ones")
            nc.vector.memset(ones[:], 1.0)
            stag = singles.tile([_I, 3 * (_C + 1)], f32, tag="stag")

            # masks, y-on-partitions (natural resize layout): msk_l[y, i, x]
            msk0 = singles.tile([128, _I, 128], bf16, tag="msk0")
            msk1 = singles.tile([64, _I, 64], bf16, tag="msk1")
            msk2 = singles.tile([32, _I, 32], bf16, tag="msk2")
            msk = {0: msk0, 1: msk1, 2: msk2}
            # assembled feature tiles [y, x, c] bf16
            sgT0 = singles.tile([128, 128, _C], bf16, tag="sgT0")
            sgT1 = singles.tile([64, 64, _C], bf16, tag="sgT1")
            sgT2 = singles.tile([32, 32, _C], bf16, tag="sgT2")
            sgT = {0: sgT0, 1: sgT1, 2: sgT2}

            # ---- interleaved per-level streams ----------------------
            # Queues: gpsimd = scribbles (4KB row-pair descs), sync/scalar =
            # feature full-row loads.  The DVE FIFO alternates one feature
            # assembly copy with one tile's mask ALU so neither stream
            # stalls the other; emission order == engine FIFO order.

            def mask_ops(li, i, st, il=None):
                # only the 2-of-k needed resize columns are added (strided)
                h, k, off = _LEVELS[li]
                src_lo = st[:, il, 0:512] if il is not None else st[:, 0, :]
                src_hi = st[:, il, 512:1024] if il is not None else st[:, 1, :]
                a = src_lo.rearrange("p (x k) -> p x k", k=k)[:, :, off : off + 2]
                b = src_hi.rearrange("p (x k) -> p x k", k=k)[:, :, off : off + 2]
                v = tmp.tile([h, h, 2], f32, tag="v")
                nc.vector.tensor_add(v[:], a, b)
                sr = tmp.tile([h, h], f32, tag="sr")
                nc.vector.tensor_add(sr[:], v[:, :, 0], v[:, :, 1])
                nc.vector.tensor_scalar(
                    out=msk[li][:, i, :], in0=sr[:], scalar1=2.0,
                    scalar2=None, op0=gt,
                )

            def feat_load_copy(li, g):
                # 16-channel staging groups; loads alternate the two HWDGE
                # rings (descriptor generation is ~1.4ns/desc and must be
                # split).  Copies are cross-assigned -- scalar copies what
                # sync loaded and the DVE copies what scalar loaded -- so a
                # copy never waits on its own engine's queue head.
                h = _LEVELS[li][0]
                sg = fR.tile([h, 16, h], f32, tag="sgR")
                deng = nc.sync if g % 2 == 0 else nc.scalar
                deng.dma_start(
                    out=sg[:],
                    in_=feats[li][16 * g : 16 * (g + 1)].rearrange(
                        "c y x -> y c x"
                    ),
                )
                dst = sgT[li][:, :, 16 * g : 16 * (g + 1)]
                srcv = sg[:].rearrange("p c x -> p x c")
                if g % 2 == 0:
                    nc.scalar.copy(dst, srcv)
                else:
                    nc.vector.tensor_copy(dst, srcv)

            def scrib_load(li, i):
                h, k, off = _LEVELS[li]
                rr = 512 // h
                st = scrib2.tile([h, 2, 512], f32, tag=f"st{li}")
                nc.gpsimd.dma_start(
                    out=st[:],
                    in_=scr[i].rearrange("(y r) x -> y r x", r=rr)[
                        :, off : off + 2, :
                    ],
                )
                return st

            # main loop: L0 features+masks with the L1/L2 scribble+mask
            # streams interleaved round-robin so they finish inside L0's
            # DMA phase instead of trailing it
            for t in range(8):
                i0 = t * 2
                st = scrib.tile([128, 2, 1024], f32, tag="st0")
                nc.gpsimd.dma_start(
                    out=st[:],
                    in_=scr[i0 : i0 + 2]
                    .rearrange("i (y k) x -> y i k x", k=4)[:, :, 1:3, :]
                    .rearrange("y i k x -> y i (k x)"),
                )
                for il in range(2):
                    i = i0 + il
                    feat_load_copy(0, 2 * t + il)
                    mask_ops(0, i, st, il=il)
                    mask_ops(1, i, scrib_load(1, i))
                    mask_ops(2, i, scrib_load(2, i))

            for li in (1, 2):
                for i in range(_I):
                    feat_load_copy(li, i)

            # ---- matmuls + cnt + staging, level order 0, 1, 2
            for li in (0, 1, 2):
                h = _LEVELS[li][0]
                acc = psum.tile([_I, _C], f32, tag="acc")
                for x in range(h):
                    nc.tensor.matmul(
                        acc[:], msk[li][:, :, x], sgT[li][:, x, :],
                        start=(x == 0), stop=(x == h - 1),
                    )
                r = singles.tile([h, _I], f32, tag=f"r{li}")
                nc.vector.reduce_sum(out=r[:], in_=msk[li][:], axis=X)
                cnt = psum.tile([_I, 1], f32, tag="cnt")
                nc.tensor.matmul(cnt[:], r[:], ones[:h, :], start=True, stop=True)
                base = li * (_C + 1)
                nc.vector.tensor_copy(stag[:, base : base + _C], acc[:])
                nc.vector.tensor_copy(stag[:, base + _C : base + _C + 1], cnt[:])

            nc.sync.dma_start(out=out_d[:], in_=stag[:])

    nc.compile()
    return nc


def _host_fallback(scr_bi, fmap_b, h, k, off):
    """Feature at argmax of the soft mask; only used when a mask is empty."""
    V = scr_bi[off::k, :][:h].astype(np.float32) + scr_bi[off + 1 :: k, :][:h]
    sr4 = V[:, off::k][:, :h] + V[:, off + 1 :: k][:, :h]
    idx = int(np.argmax(np.float32(0.25) * sr4))
    y, x = divmod(idx, h)
    return fmap_b[:, y, x]


def kernel(feat0, feat1, feat2, scribbles):
    import sys

    for p in ("/opt/trn_rl_repo", "/opt/pypackages"):
        if p not in sys.path:
            sys.path.append(p)
    from concourse.bass_utils import run_bass_kernel_spmd

    feat0 = np.asarray(feat0, dtype=np.float32)
    feat1 = np.asarray(feat1, dtype=np.float32)
    feat2 = np.asarray(feat2, dtype=np.float32)
    scribbles = np.asarray(scribbles, dtype=np.float32)

    nc = _build_nc()
    in_maps = [
        {
            "feat0": np.ascontiguousarray(feat0[b]),
            "feat1": np.ascontiguousarray(feat1[b]),
            "feat2": np.ascontiguousarray(feat2[b]),
            "scribbles": np.ascontiguousarray(scribbles[b]),
        }
        for b in range(_B)
    ]
    res = run_bass_kernel_spmd(nc, in_maps, core_ids=list(range(_B)))
    raw = np.stack([res.results[b]["out"] for b in range(_B)])  # [B, I, 3*257]
    raw = raw.reshape(_B, _I, 3, _C + 1)
    ssum = raw[..., :_C].astype(np.float32)  # [B, I, 3, C]
    cnt = raw[..., _C].astype(np.float32)  # [B, I, 3]

    mean = ssum / np.maximum(cnt, np.float32(1.0))[..., None]

    if (cnt == 0).any():  # never for non-degenerate inputs
        fm = [feat0, feat1, feat2]
        for b, i, li in zip(*np.nonzero(cnt == 0)):
            h, k, off = _LEVELS[li]
            mean[b, i, li] = _host_fallback(scribbles[b, i], fm[li][b], h, k, off)

    out = (mean[:, :, 0] + mean[:, :, 1] + mean[:, :, 2]) / np.float32(3.0)
    return out.astype(np.float32)
